# revision 1
# baseline (speedup 1.0000x reference)
"""Trainium2 Bass kernel for nn_CCALoss (CLIP loss + concept BCE + Jaccard-softmax KL).

Sharding: data-parallel over batch rows. Each of the 8 cores receives B/8 = 64
rows of every [B, *] tensor, plus the full transposed concept matrix (the
"all-gather" is done host-side since the kernel receives full inputs anyway).

Key algebraic rewrites (w = relu(medical_concepts) is binary {0,1}):
    inter[i,j] = sum_c min(w_i, w_j) = w_i . w_j            -> PE matmul
    union[i,j] = s_i + s_j - inter[i,j],  s_i = sum_c w_i
    s_j - inter[i,j] = sum_c (1 - w_i[c]) * w_j[c]          -> PE matmul with
                                                               complement weights
so psum_u = (1-w_shard).T @ w_full needs no extra rank-1 terms; s_i rides the
union clamp as a per-partition tensor_scalar operand.

Numerics: all softmax max-subtractions are dropped — inputs are bounded
(logits ~ N(0,9) -> exp <= e^~15, sim/T <= 1/0.07 = 14.3), well within f32.

Layouts: all [64, 512] row-major work is reshaped to a "split" [128, 256]
layout (row i cols 0:256 -> partition i; cols 256:512 -> partition 64+i):
DVE/ACT cost scales with free bytes per partition, so this halves op time.
Row reductions combine partition pairs (i, 64+i) with one tiny op.

Engines: PE for contractions, DVE for float elementwise/reduces + the tail
scalar algebra, ACT for transcendentals, GpSimd for the BCE elementwise chain.
Raw Bass with standalone wait_ge sync (this toolchain's walrus allows at most
one attached semaphore wait per instruction and cannot encode
tensor_tensor_reduce).

Each core writes 5 partial sums (clip LSE sum, clip diag sum, KL sum, BCE sum,
mask count); the host combines them into the scalar loss.
"""

from contextlib import ExitStack

import numpy as np

import concourse.bass as bass
import concourse.mybir as mybir
from concourse.bass_utils import run_bass_kernel_spmd

F8NP = mybir.dt.np(mybir.dt.float8e4)

AF = mybir.ActivationFunctionType
ALU = mybir.AluOpType
AX = mybir.AxisListType

B = 512  # batch
C = 256  # concepts
M = 8  # cores
R = B // M  # rows per core = 64
P = 128
TEMP = 0.07
CONCEPT_WEIGHT = 0.5
CONCEPT_SIM_WEIGHT = 0.3

F32 = mybir.dt.float32
I8 = mybir.dt.int8
BF16 = mybir.dt.bfloat16
F8 = mybir.dt.float8e4

H = 256  # split-layout free size (B/2)
HC = 128  # split-layout free size for [R, C] tensors (C/2)

# wpack cols (fp8): [(1-w_shard.T) k0 (64) | k1 (64) |
#   w_full.T k0h0 (256) + ones col | k0h1 + ones | k1h0 + ones | k1h1 + ones |
#   w_shard.T k0 (64) | k1 (64)]
# The ones column after each w_full half accumulates 256 - s_i into psum_u's
# extra column (lhsT is the complement), giving s_i without extra matmuls.
WH = H + 1  # 257
WPK = 2 * R + 4 * WH + 2 * R  # 1284
# bpack cols (bytes): [cl split f32 (128*4) | mc split i8 (128) | dmask f32 (64*4)]
BPK = HC * 4 + HC + R * 4  # 896
# fpack cols (f32): [lpit (512) | cis split (256)]
FPK = B + H  # 768


def _build():
    nc = bass.Bass()

    wpack = nc.declare_dram_parameter("wpack", [P, WPK], F8, isOutput=False)
    fpack = nc.declare_dram_parameter("fpack", [P, FPK], F32, isOutput=False)
    bpack = nc.declare_dram_parameter("bpack", [P, BPK], I8, isOutput=False)
    out_p = nc.declare_dram_parameter("partials", [P, 8 + HC], F32, isOutput=True)

    ctx = ExitStack()

    def sb(shape, dtype, name):
        return ctx.enter_context(nc.sbuf_tensor(name, shape, dtype))

    def ps(shape, name):
        return ctx.enter_context(nc.psum_tensor(name, shape, F32))

    with ctx:
        # ---------------- tiles ----------------
        wpack_t = sb([P, WPK], F8, "wpack_t")
        bpack_t = sb([P, BPK], I8, "bpack_t")
        fpack_t = sb([P, FPK], F32, "fpack_t")

        STW = 8 + HC  # stats + raw BCE masked-loss tile (host reduces it)
        stats = sb([P, STW], F32, "stats")

        t_w = sb([P, HC], F32, "t_w")
        maskv = sb([P, HC], F32, "maskv")
        s128 = sb([P, 1], F32, "s128")

        union_c = sb([P, H], F32, "union_c")
        rec = sb([P, H], F32, "rec")
        sim_t = sb([P, H], F32, "sim_t")
        e_t = sb([P, H], F32, "e_t")
        diff2 = sb([P, H], F32, "diff2")
        prod = sb([P, H], F32, "prod")
        d_red = sb([P, 1], F32, "d_red")
        ecis = sb([P, H], F32, "ecis")
        sc_h = sb([P, 1], F32, "sc_h")
        se_h = sb([P, 1], F32, "se_h")
        eclip = sb([P, B], F32, "eclip")
        lnpack = sb([P, 2], F32, "lnpack")  # col0: sclip; col1: sum_e | scis
        dgm = sb([P, R], F32, "dgm")
        csT_t = sb([P, H], F32, "csT_t")
        rec_se = sb([R, 1], F32, "rec_se")
        rec128 = sb([P, 1], F32, "rec128")

        ab = sb([P, HC], F32, "ab")
        en = sb([P, HC], F32, "en")
        lp = sb([P, HC], F32, "lp")
        rl = sb([P, HC], F32, "rl")
        sp = sb([P, HC], F32, "sp")
        q = sb([P, HC], F32, "q")
        r_t = sb([P, HC], F32, "r_t")
        rm = sb([P, HC], F32, "rm")
        msc = sb([P, HC], F32, "msc")
        bsc = sb([P, HC], F32, "bsc")
        dsc = sb([P, R], F32, "dsc")

        psum_i = ps([P, H], "psum_i")
        psum_u = ps([P, 2 * H], "psum_u")  # bank-row padded; only 0:WH used

        # views
        clip2 = fpack_t[:, 0:B]
        cs = fpack_t[:, B : B + H]
        cls = bpack_t[:, 0 : HC * 4].bitcast(F32)
        mcs = bpack_t[:, HC * 4 : HC * 4 + HC]
        dmask_v = bpack_t[:, HC * 4 + HC : BPK].bitcast(F32)

        def comp_k(k):  # [128, 64] fp8, complement weights
            return wpack_t[:, k * R : (k + 1) * R]

        def wTk(k, h, n):  # [128, n] fp8: w_full chunk k, column half h
            c0 = 2 * R + (2 * k + h) * WH
            return wpack_t[:, c0 : c0 + n]

        def wTs_k(k):  # [128, 64] fp8
            c0 = 2 * R + 4 * WH
            return wpack_t[:, c0 + k * R : c0 + (k + 1) * R]

        # ---------------- planner ----------------
        # stats is tracked per column so disjoint column writers don't serialize
        class _Col:
            def __init__(self, j):
                self.j = j
        stats_cols = [_Col(j) for j in range(9)]  # col index 8 = rm region
        # lnpack regions: 0 = col0 (sclip), 1 = col1 rows 0:64 (sum_e),
        # 2 = col1 rows 64:128 (scis)
        ln_regs = [_Col(10 + j) for j in range(3)]
        plan = []

        def op(eng, fn, reads, writes):
            plan.append((eng, fn, tuple(reads), tuple(writes)))

        dma_loads = [
            ("d_w", wpack_t, lambda: wpack[:, :]),
            ("d_f", fpack_t, lambda: fpack[:, :]),
        ]
        # bpack is issued from the ACT sequencer in parallel with SP's issues
        v_dma_loads = [
            ("d_b", bpack_t, lambda: bpack[:, :]),
        ]

        V, A, T, G = "V", "A", "T", "P"

        # --- V: constants
        op(V, lambda: nc.vector.memset(stats[:, :], 0.0), [], list(stats_cols))

        # --- PE: (s_j - inter | 256 - s_i) first, then inter.
        # fp8 weights are used directly from the DMA'd pack - no conversion.
        for h in (0, 1):
            lo, hi = h * R, (h + 1) * R
            op(T, lambda h=h, lo=lo, hi=hi: nc.tensor.matmul(
                psum_u[lo:hi, 0:WH], comp_k(0), wTk(0, h, WH), start=True, stop=False,
                skip_group_check=True), [wpack_t], [psum_u])
        for h in (0, 1):
            lo, hi = h * R, (h + 1) * R
            op(T, lambda h=h, lo=lo, hi=hi: nc.tensor.matmul(
                psum_u[lo:hi, 0:WH], comp_k(1), wTk(1, h, WH), start=False, stop=True,
                skip_group_check=True), [wpack_t], [psum_u])
        for h in (0, 1):
            lo, hi = h * R, (h + 1) * R
            op(T, lambda h=h, lo=lo, hi=hi: nc.tensor.matmul(
                psum_i[lo:hi, :], wTs_k(0), wTk(0, h, H), start=True, stop=False),
               [wpack_t], [psum_i])
            op(T, lambda h=h, lo=lo, hi=hi: nc.tensor.matmul(
                psum_i[lo:hi, :], wTs_k(1), wTk(1, h, H), start=False, stop=True),
               [wpack_t], [psum_i])

        # s128 = 256 - psum_u[:, 256]  (per-partition s_i, into SBUF)
        op(V, lambda: nc.vector.tensor_scalar(
            out=s128[:, :], in0=psum_u[:, H : H + 1], scalar1=-1.0, scalar2=256.0,
            op0=ALU.mult, op1=ALU.add), [psum_u], [s128])

        # --- Pool: BCE elementwise chain + diag select + csT
        op(G, lambda: nc.gpsimd.tensor_scalar(
            out=t_w[:, :], in0=mcs, scalar1=0, scalar2=None, op0=ALU.max),
           [bpack_t], [t_w])
        op(G, lambda: nc.gpsimd.tensor_scalar(
            out=maskv[:, :], in0=mcs, scalar1=1, scalar2=1.0,
            op0=ALU.add, op1=ALU.min), [bpack_t], [maskv])


        # --- ACT: BCE transcendentals early, then s-tiles, then the exps
        op(A, lambda: nc.scalar.activation(out=ab[:, :], in_=cls, func=AF.Abs),
           [bpack_t], [ab])
        op(A, lambda: nc.scalar.activation(out=rl[:, :], in_=cls, func=AF.Relu),
           [bpack_t], [rl])
        op(A, lambda: nc.scalar.activation(out=en[:, :], in_=ab[:, :], func=AF.Exp, scale=-1.0),
           [ab], [en])

        op(G, lambda: nc.gpsimd.tensor_mul(q[:, :], cls, t_w[:, :]),
           [bpack_t, t_w], [q])

        # --- DVE: mask count; ACT: softplus ln
        op(A, lambda: nc.scalar.activation(out=lp[:, :], in_=en[:, :], func=AF.Ln, bias=1.0),
           [en], [lp])


        # --- Pool: csT (before diff2 needs it), then softplus assembly, diag
        op(G, lambda: nc.gpsimd.tensor_mul(dgm[:, :], fpack_t[:, 0:R], dmask_v),
           [fpack_t, bpack_t], [dgm])
        op(G, lambda: nc.gpsimd.tensor_scalar(
            out=csT_t[:, :], in0=cs, scalar1=TEMP, scalar2=None, op0=ALU.mult),
           [fpack_t], [csT_t])
        op(G, lambda: nc.gpsimd.tensor_add(sp[:, :], rl[:, :], lp[:, :]),
           [rl, lp], [sp])
        op(G, lambda: nc.gpsimd.tensor_sub(r_t[:, :], sp[:, :], q[:, :]),
           [sp, q], [r_t])
        op(G, lambda: nc.gpsimd.tensor_mul(stats[:, 8 : 8 + HC], r_t[:, :], maskv[:, :]),
           [r_t, maskv, stats_cols[8]], [stats_cols[8]])


        # --- ACT: eclip, then esim as soon as sim lands, then ecis
        op(A, lambda: nc.scalar.activation(
            out=eclip[:, :], in_=clip2, func=AF.Exp, accum_out=lnpack[:, 0:1]),
           [fpack_t], [eclip, ln_regs[0]])

        # --- Jaccard similarity chain (DVE)
        op(V, lambda: nc.vector.tensor_scalar(
            out=union_c[:, :], in0=psum_u[:, 0:H], scalar1=s128[:, :], scalar2=0.5,
            op0=ALU.add, op1=ALU.max), [psum_u, s128], [union_c])
        op(V, lambda: nc.vector.reciprocal(out=rec[:, :], in_=union_c[:, :]),
           [union_c], [rec])
        op(V, lambda: nc.vector.tensor_mul(sim_t[:, :], psum_i[:, :], rec[:, :]),
           [psum_i, rec], [sim_t])
        op(V, lambda: nc.vector.tensor_sub(diff2[:, :], sim_t[:, :], csT_t[:, :]),
           [sim_t, csT_t], [diff2])
        op(A, lambda: nc.scalar.activation(
            out=e_t[:, :], in_=sim_t[:, :], func=AF.Exp, scale=1.0 / TEMP,
            accum_out=se_h[:, :]), [sim_t], [e_t, se_h])
        op(A, lambda: nc.scalar.activation(
            out=lnpack[0:R, 1:2], in_=se_h[0:R, :], func=AF.Identity, bias=se_h[R:P, :]),
           [se_h], [ln_regs[1]])
        op(A, lambda: nc.scalar.activation(
            out=ecis[:, :], in_=cs, func=AF.Exp, accum_out=sc_h[:, :]),
           [fpack_t], [ecis, sc_h])
        op(A, lambda: nc.scalar.activation(
            out=lnpack[R:P, 1:2], in_=sc_h[0:R, :], func=AF.Identity, bias=sc_h[R:P, :]),
           [sc_h], [ln_regs[2]])
        # Ln writes straight into stats cols {0,5} (strided free AP):
        # col0 = ln(sclip); col5 rows 0:64 = ln(sum_e), rows 64:128 = ln(scis)
        op(A, lambda: nc.scalar.activation(
            out=bass.AP(tensor=stats, offset=0, ap=[[STW, P], [5, 2]]),
            in_=lnpack[:, :], func=AF.Ln),
           [ln_regs[0], ln_regs[1], ln_regs[2], stats_cols[0], stats_cols[5]],
           [stats_cols[0], stats_cols[5]])
        op(V, lambda: nc.vector.reduce_sum(out=stats[:, 4:5], in_=maskv[:, :], axis=AX.X),
           [maskv, stats_cols[4]], [stats_cols[4]])
        op(V, lambda: nc.vector.reduce_sum(out=stats[:, 1:2], in_=dgm[:, :], axis=AX.X),
           [dgm, stats_cols[1]], [stats_cols[1]])
        op(V, lambda: nc.vector.tensor_mul(prod[:, :], e_t[:, :], diff2[:, :]),
           [e_t, diff2], [prod])
        op(V, lambda: nc.vector.reciprocal(out=rec_se[:, :], in_=lnpack[0:R, 1:2]),
           [ln_regs[1]], [rec_se])
        op(V, lambda: nc.vector.tensor_copy(out=rec128[0:R, :], in_=rec_se[:, :]),
           [rec_se], [rec128])
        op(V, lambda: nc.vector.tensor_copy(out=rec128[R:P, :], in_=rec_se[:, :]),
           [rec_se], [rec128])
        op(V, lambda: nc.vector.reduce_sum(out=d_red[:, :], in_=prod[:, :], axis=AX.X),
           [prod], [d_red])
        op(V, lambda: nc.vector.tensor_scalar(
            out=stats[:, 2:3], in0=d_red[:, :], scalar1=rec128[:, :], scalar2=None,
            op0=ALU.mult), [d_red, rec128, stats_cols[2]], [stats_cols[2]])

        # ---------------- two-pass emission ----------------
        last_writer = {}
        for name, tile_, _src in dma_loads + v_dma_loads:
            last_writer[id(tile_)] = (name, 16)
        counts = {"V": 0, "A": 0, "T": 0, "P": 0}
        waits_needed = []
        for eng, fn, reads, writes in plan:
            need = {}
            for tset_i, tset in enumerate((reads, writes)):
                for tile_ in tset:
                    lw = last_writer.get(id(tile_))
                    assert tset_i == 1 or lw is not None, (
                        f"plan not topological: read of unwritten tile {tile_}"
                    )
                    if lw is not None:
                        k, t = lw
                        if need.get(k, 0) < t:
                            need[k] = t
            waits_needed.append(sorted(need.items()))
            counts[eng] += 1
            for tile_ in writes:
                last_writer[id(tile_)] = (eng, counts[eng])
        stats_finals = {}
        cnt2 = {"V": 0, "A": 0, "T": 0, "P": 0}
        for eng, fn, reads, writes in plan:
            cnt2[eng] += 1
            for tile_ in writes:
                if tile_ in stats_cols:
                    stats_finals[eng] = cnt2[eng]

        with ExitStack() as semctx:
            sems = {}
            for k in ("V", "A", "T", "P"):
                sems[k] = semctx.enter_context(nc.semaphore(f"sem_{k}"))
            for name, _t, _src in dma_loads + v_dma_loads:
                sems[name] = semctx.enter_context(nc.semaphore(f"sem_{name}"))
            out_dma_sem = semctx.enter_context(nc.semaphore("sem_out"))

            engines = {"V": nc.vector, "A": nc.scalar, "T": nc.tensor, "P": nc.gpsimd}
            observed = {k: {} for k in ("V", "A", "T", "P")}

            def emit_for(eng):
                for (e, fn, reads, writes), need in zip(plan, waits_needed):
                    if e != eng:
                        continue
                    obs = observed[eng]
                    for k, t in need:
                        if obs.get(k, 0) < t:
                            engines[eng].wait_ge(sems[k], t)
                            obs[k] = t
                    instr = fn()
                    instr.then_inc(sems[eng], 1)

            with nc.Block(no_gpsimd_drain=True) as block:

                @block.sync
                def _(sync):
                    for name, tile_, src in dma_loads:
                        sync.dma_start(out=tile_[:], in_=src()).then_inc(
                            sems[name], 16
                        )
                    for eng_k, tick in sorted(stats_finals.items()):
                        sync.wait_ge(sems[eng_k], tick)
                    sync.dma_start(out=out_p[:, :], in_=stats[:, :]).then_inc(
                        out_dma_sem, 16
                    )

                @block.vector
                def _(vector):
                    emit_for("V")

                @block.scalar
                def _(scalar):
                    for name, tile_, src_ in v_dma_loads:
                        scalar.dma_start(out=tile_[:], in_=src_()).then_inc(
                            sems[name], 16
                        )
                    emit_for("A")

                @block.tensor
                def _(tensor):
                    emit_for("T")

                @block.gpsimd
                def _(gpsimd):
                    emit_for("P")

    return nc


_NC = None


def _get_nc():
    global _NC
    if _NC is None:
        _NC = _build()
    return _NC


def _split(x):
    """[64, 2h] -> [128, h]: row i cols 0:h -> partition i; cols h:2h -> 64+i."""
    h = x.shape[1] // 2
    return np.concatenate([x[:, :h], x[:, h:]], axis=0)


def make_in_maps(inputs):
    lpi = np.ascontiguousarray(np.asarray(inputs["logits_per_image"], dtype=np.float32))
    lpt = np.ascontiguousarray(np.asarray(inputs["logits_per_text"], dtype=np.float32))
    cl = np.ascontiguousarray(np.asarray(inputs["concepts_logits"], dtype=np.float32))
    cis = np.ascontiguousarray(
        np.asarray(inputs["concepts_image_similarity"], dtype=np.float32)
    )
    mc = np.ascontiguousarray(np.asarray(inputs["medical_concepts"], dtype=np.int32))

    w8T = np.maximum(mc.T, 0).astype(np.int8)  # [C, B] binary
    dmask = np.zeros((P, R), dtype=np.float32)
    dmask[np.arange(P), np.arange(P) % R] = 1.0
    dmask8 = dmask.view(np.int8).reshape(P, R * 4)

    in_maps = []
    for i in range(M):
        r0 = i * R
        sl = slice(r0, r0 + R)

        ws8 = np.maximum(mc[sl].T, 0).astype(np.int8)  # [C, R] binary
        comp8 = (1 - ws8).astype(np.int8)
        onec = np.ones((P, 1), dtype=np.int8)
        wpk = np.concatenate(
            [comp8[0:P, :], comp8[P:C, :],
             w8T[0:P, 0:H], onec, w8T[0:P, H:B], onec,
             w8T[P:C, 0:H], onec, w8T[P:C, H:B], onec,
             ws8[0:P, :], ws8[P:C, :]], axis=1
        ).astype(F8NP)  # [128, 1284] fp8

        cls = _split(cl[sl])  # [128, 128] f32
        mcs = _split(mc[sl]).astype(np.int8)  # [128, 128] i8
        bpk = np.concatenate(
            [cls.view(np.int8).reshape(P, HC * 4), mcs, dmask8], axis=1
        )  # [128, 896] i8

        lpit = np.concatenate(
            [np.roll(lpi[sl], -r0, axis=1), np.roll(lpt[sl], -r0, axis=1)], axis=0
        )  # [128, 512]
        fpk = np.concatenate([lpit, _split(cis[sl])], axis=1)  # [128, 768] f32

        in_maps.append(
            {
                "wpack": np.ascontiguousarray(wpk),
                "fpack": np.ascontiguousarray(fpk),
                "bpack": np.ascontiguousarray(bpk),
            }
        )
    return in_maps


def combine_partials(per_core_partials):
    S = np.zeros(8)
    kl_extra = 0.0
    for p in per_core_partials:
        a = np.asarray(p, dtype=np.float64).reshape(P, 8 + HC)
        S += a[:, 0:8].sum(axis=0)
        S[3] += a[:, 8:].sum()
        # col5 rows 0:64 = +ln(sum_e) (subtract), rows 64:128 = +ln(scis) (add)
        kl_extra += a[R:P, 5].sum() - a[0:R, 5].sum()
    clip_loss = (S[0] - S[1]) / (2.0 * B)
    concept_loss = S[3] / (S[4] + 1e-8)
    concept_sim_loss = (S[2] / TEMP + kl_extra) / B
    total = clip_loss + CONCEPT_WEIGHT * concept_loss + CONCEPT_SIM_WEIGHT * concept_sim_loss
    return np.float32(total)


def run_spmd(inputs, **kwargs):
    in_maps = make_in_maps(inputs)
    return run_bass_kernel_spmd(_get_nc(), in_maps, core_ids=list(range(M)), **kwargs)


def kernel(**inputs):
    res = run_spmd(inputs)
    return combine_partials([r["partials"] for r in res.results])

